# revision 1
# baseline (speedup 1.0000x reference)
"""Conv2D-KAN Trainium2 kernel (8-core data-parallel SPMD).

Formulation
-----------
The reference computes, per 3x3 patch (N = B*30*30 patches, in_size = 288):
    out[n,o] = sum_{i,k} sb[n,i,k] * (spline_kernel*scale)[i,k,o]
             + silu(xf) @ scale_factor + biases
where sb is a cubic B-spline basis (8 funcs) over a uniform grid
(knots t_r = -2.2 + 0.4 r, r = 0..11, h = 0.4).

Key identities:
 1. Basis values depend only on the underlying *pixel*, not the patch
    (patch extraction is a gather), so features are computed per pixel
    (8x less elementwise work than per-patch).
 2. Uniform cubic B-splines decompose over truncated powers:
        B_k(x) = (1/6) sum_{m=0..4} cm_m T_{k+m}(x), cm = [1,-4,6,-4,1]
        T_r(x) = min(relu((x - t_r)/h), 11-r)^3
    The clamp at 11-r makes every B_k *exactly* zero outside the grid
    (integer cancellation), matching the reference's out-of-range
    behaviour without masks, and T_11 == 0 so only r = 0..10 exist.
 3. The whole op is then a 3x3 convolution with 128 filters over
    pixel-feature channels, done as accumulating 128-K matmuls into
    PSUM banks of [128 filters, 450 patches].

Two modes:
 * "fp32"  — features are the 11 truncated cubes + silu per channel
             (384 = 3x128 K-chunks per offset, 27 matmuls per bank),
             blending folded into the weights. Full fp32 matmuls
             (4 cyc/row). Max rel err ~1e-5.
 * "basis" — the blending T -> B_k happens on DVE in fp32 (exact), so
             the matmul operands are the well-conditioned basis values
             (<= 4) and the matmuls run in float32r (TF32-like, 1-pass,
             ~1.4 cyc/row). 8 basis + silu -> 2x128 + 32 K-chunks per
             offset, 27 matmuls per bank. Rel err ~ a few 1e-5.

Each core processes 4 images; output [128, 3600] per core is
transposed on host.
"""

import sys

sys.path.insert(0, "/opt/trn_rl_repo")

import numpy as np

N_CORES = 8
B, HH, WW, C = 32, 32, 32, 32
F = 128
KH = KW = 3
HO, WO = HH - KH + 1, WW - KW + 1          # 30, 30
BPC = B // N_CORES                          # images per core = 4
PIX = HH * WW                               # 1024 pixels per image
NPC = BPC * HO * WO                         # 3600 patches per core
NBANK = 2 * BPC                             # 8 psum banks
BANKN = NPC // NBANK                        # 450
HGRID = 0.4
T0 = -2.2                                   # first knot
NR = 11                                     # truncated-cube features
NFEAT = 12                                  # + silu
NMM = 27                                    # matmuls per bank (both modes)

MODE = "fp32"  # "fp32" | "basis"

_cache = {}


def _build_program(mode):
    import concourse.bacc as bacc
    import concourse.mybir as mybir
    import concourse.tile as tile

    f32 = mybir.dt.float32
    f32r = mybir.dt.float32r
    AF = mybir.ActivationFunctionType
    basis = mode == "basis"

    nch = NMM + 2 if basis else NMM
    nc = bacc.Bacc("TRN2", target_bir_lowering=False, debug=False)
    xt = nc.dram_tensor("xt", [C, BPC * PIX], f32, kind="ExternalInput").ap()
    # weights: [128 partitions, nch * F] -> one contiguous DMA
    wt = nc.dram_tensor("wt", [128, nch * F], f32, kind="ExternalInput").ap()
    consts = nc.dram_tensor("consts", [128, 8], f32, kind="ExternalInput").ap()
    y = nc.dram_tensor("y", [F, NPC], f32, kind="ExternalOutput").ap()

    with tile.TileContext(nc) as tc:
        with (
            tc.tile_pool(name="wp", bufs=1) as wp,
            tc.tile_pool(name="cp", bufs=1) as cp,
            tc.tile_pool(name="fp", bufs=3) as fp,
            tc.tile_pool(name="sp", bufs=3) as sp,
            tc.tile_pool(name="op", bufs=1) as op_,
            tc.tile_pool(name="pp", bufs=4, space="PSUM") as pp,
        ):
            ct = cp.tile([128, 8], f32)
            nc.scalar.dma_start(ct[:], consts[:])

            # warm up the ACT table set (silu's set also carries relu /
            # copy / identity / square fillers) so the ~1.3us table load
            # happens before the first feature tile is ready.
            warm = cp.tile([1, 1], f32, tag="warm")
            nc.scalar.activation(warm[:], ct[:1, :1], AF.Silu)

            # image 0's first feature tile: its four replica DMAs split
            # across BOTH HWDGE queues ahead of all other traffic, so
            # the last completion semaphore (which lags ~2.5us behind
            # the data) lands as early as possible.
            ft00 = None
            if not basis:
                ft00 = fp.tile([128, PIX], f32, tag="f0")
                eng = [nc.sync, nc.scalar, nc.sync, nc.scalar]
                for rep in range(4):
                    eng[rep].dma_start(
                        ft00[32 * rep:32 * rep + 32], xt[:, 0:PIX])

            if basis:
                wbig = wp.tile([128, nch * F], f32, tag="wbig")
                nc.gpsimd.dma_start(wbig[:], wt[:])
                wrbig = wp.tile([128, NMM * F], f32r, tag="wrbig")
                nc.vector.tensor_copy(wrbig[:], wbig[:, :NMM * F])
                wtiles = [wrbig[:, i * F:(i + 1) * F] for i in range(NMM)]
                Ma = wbig[:, NMM * F:(NMM + 1) * F]
                Mb = wbig[:, (NMM + 1) * F:(NMM + 2) * F]
            else:
                # weights split into two tiles so the first 9 matmuls
                # (t-major order: all offsets of feature tile 0) only
                # depend on a small fast transfer; the big remainder
                # loads on the gpsimd queue in parallel.
                wA = wp.tile([128, 9 * F], f32, tag="wA")
                nc.scalar.dma_start(wA[:], wt[:, :9 * F])
                wB = wp.tile([128, 18 * F], f32, tag="wB")
                nc.gpsimd.dma_start(wB[:], wt[:, 9 * F:])
                wtiles = [wA[:, i * F:(i + 1) * F] for i in range(9)] + \
                         [wB[:, i * F:(i + 1) * F] for i in range(18)]

            out_t = op_.tile([F, NPC], f32)

            def banks(im, mk_rhs):
                for half in range(2):
                    ps = pp.tile([F, BANKN], f32, tag="ps")
                    k = 0
                    # t-major: the first 9 matmuls only need feature
                    # tile 0, so PE starts before tiles 1/2 are built
                    for t in range(3):
                        for off in range(KH * KW):
                            di, dj = divmod(off, KW)
                            h0 = half * 15 + di
                            lhsT, rhs = mk_rhs(off, t, h0, dj)
                            nc.tensor.matmul(
                                ps[:], lhsT, rhs,
                                start=(k == 0), stop=(k == NMM - 1),
                            )
                            k += 1
                    s = (im * 2 + half) * BANKN
                    nc.scalar.activation(
                        out_t[:, s:s + BANKN], ps[:], AF.Identity,
                        bias=ct[:, 6:7], scale=1.0,
                    )
                    nc.sync.dma_start(y[:, s:s + BANKN], out_t[:, s:s + BANKN])

            for im in range(BPC):
                sl = slice(im * PIX, (im + 1) * PIX)
                if basis:
                    # --- T tiles (same r-major 4r x 32c layout as fp32 mode)
                    Ts = []
                    for t in range(3):
                        T = fp.tile([128, PIX], f32, tag=f"T{t}")
                        for rep in range(4):
                            nc.sync.dma_start(
                                T[32 * rep:32 * rep + 32], xt[:, sl])
                        nc.scalar.activation(
                            T[:], T[:], AF.Relu,
                            bias=ct[:, t:t + 1], scale=1.0 / HGRID)
                        nc.vector.tensor_scalar_min(
                            T[:], T[:], ct[:, 3 + t:4 + t])
                        sq = sp.tile([128, PIX], f32, tag="sq")
                        nc.scalar.activation(sq[:], T[:], AF.Square)
                        nc.vector.tensor_mul(T[:], sq[:], T[:])
                        Ts.append(T)
                    # --- combine B_k = sum_m cm_m T_{k+m} on PE:
                    # two banded constant matrices contract the r dim
                    # (engines cannot read shifted partition windows).
                    Bviews = []
                    for g in range(2):
                        Bt = fp.tile([128, PIX], f32r, tag=f"B{g}")
                        for hf in range(2):
                            hs = slice(hf * 512, (hf + 1) * 512)
                            bp = pp.tile([128, 512], f32, tag="psB")
                            nc.tensor.matmul(bp[:], Ma, Ts[g][:, hs],
                                             start=True, stop=False)
                            nc.tensor.matmul(bp[:], Mb, Ts[g + 1][:, hs],
                                             start=False, stop=True)
                            nc.scalar.activation(Bt[:, hs], bp[:], AF.Copy)
                        Bviews.append(
                            Bt[:].rearrange("p (h w) -> p h w", w=WW))
                    # --- silu ---
                    xs = sp.tile([32, PIX], f32, tag="xs")
                    nc.sync.dma_start(xs[:], xt[:, sl])
                    SL = fp.tile([32, PIX], f32r, tag="SL")
                    nc.scalar.activation(SL[:], xs[:], AF.Silu)
                    slv = SL[:].rearrange("p (h w) -> p h w", w=WW)

                    def mk_rhs(off, t, h0, dj, _B=Bviews, _s=slv, _w=wtiles):
                        if t < 2:
                            return (_w[off * 3 + t],
                                    _B[t][:, h0:h0 + 15, dj:dj + WO])
                        return (_w[off * 3 + 2][0:32],
                                _s[:, h0:h0 + 15, dj:dj + WO])

                    banks(im, mk_rhs)
                else:
                    views = []
                    dma_eng = [nc.sync, nc.scalar, nc.sync]
                    for t in range(3):
                        if im == 0 and t == 0:
                            ft = ft00
                        else:
                            ft = fp.tile([128, PIX], f32, tag=f"f{t}")
                            for rep in range(4):
                                dma_eng[t].dma_start(
                                    ft[32 * rep:32 * rep + 32], xt[:, sl])
                        nsp = 128 if t < 2 else 96
                        nc.scalar.activation(
                            ft[:nsp], ft[:nsp], AF.Relu,
                            bias=ct[:nsp, t:t + 1], scale=1.0 / HGRID)
                        if t == 2:
                            nc.scalar.activation(
                                ft[96:128], ft[96:128], AF.Silu)
                        nc.vector.tensor_scalar_min(
                            ft[:nsp], ft[:nsp], ct[:nsp, 3 + t:4 + t])
                        sq = sp.tile([128, PIX], f32, tag="sq")
                        nc.vector.tensor_mul(sq[:nsp], ft[:nsp], ft[:nsp])
                        nc.vector.tensor_mul(ft[:nsp], sq[:nsp], ft[:nsp])
                        views.append(
                            ft[:].rearrange("p (h w) -> p h w", w=WW))

                    def mk_rhs(off, t, h0, dj, _v=views, _w=wtiles):
                        return (_w[t * 9 + off],
                                _v[t][:, h0:h0 + 15, dj:dj + WO])

                    banks(im, mk_rhs)

    nc.compile()
    return nc


def _prep_fp32(spline_kernel, scale_factor):
    """Truncated-power-folded weights, r-major (r, c) K layout."""
    w = spline_kernel.astype(np.float64) * scale_factor.astype(np.float64)[:, None, :]
    cm = np.array([1.0, -4.0, 6.0, -4.0, 1.0], np.float64) / 6.0
    Wp = np.zeros((KH * KW, NFEAT, C, F), np.float64)
    wr = w.reshape(KH * KW, C, 8, F)
    for r in range(NR):
        for m in range(5):
            k = r - m
            if 0 <= k < 8:
                Wp[:, r] += wr[:, :, k] * cm[m]
    Wp[:, NR] = scale_factor.astype(np.float64).reshape(KH * KW, C, F)
    Wt = Wp.reshape(KH * KW, 3, 128, F)
    # device chunk order is t-major: chunk index = t*9 + off
    return np.ascontiguousarray(Wt.transpose(1, 0, 2, 3)).reshape(NMM, 128, F)


def _prep_basis(spline_kernel, scale_factor):
    """Raw spline weights /6, (4k x 32c) K layout + silu chunks."""
    w6 = (spline_kernel.astype(np.float64)
          * scale_factor.astype(np.float64)[:, None, :]) / 6.0
    w6 = w6.reshape(KH * KW, C, 8, F)
    sf = scale_factor.astype(np.float64).reshape(KH * KW, C, F)
    Wt = np.zeros((NMM + 2, 128, F), np.float64)
    for off in range(KH * KW):
        for g in range(2):
            blk = w6[off, :, 4 * g:4 * g + 4]            # (32c, 4k, F)
            Wt[off * 3 + g] = blk.transpose(1, 0, 2).reshape(128, F)
        Wt[off * 3 + 2, 0:32] = sf[off]
    # banded combine matrices: B[p_out] = sum_in M[p_in, p_out] T[p_in]
    cm = np.array([1.0, -4.0, 6.0, -4.0, 1.0])
    pin = np.arange(128)[:, None]
    pout = np.arange(128)[None, :]
    same_c = (pin % 32) == (pout % 32)
    for j, base in ((NMM, 0), (NMM + 1, 4)):
        m = base + pin // 32 - pout // 32
        val = np.where((m >= 0) & (m <= 4) & same_c, cm[np.clip(m, 0, 4)], 0.0)
        Wt[j] = val
    return Wt


def _prep_static(mode, spline_kernel, scale_factor, kan_bias, conv_bias):
    if mode == "basis":
        Wt = _prep_basis(spline_kernel, scale_factor)
    else:
        Wt = _prep_fp32(spline_kernel, scale_factor)
    nch = Wt.shape[0]
    wt = np.ascontiguousarray(
        Wt.transpose(1, 0, 2).reshape(128, nch * F), np.float32)

    consts = np.zeros((128, 8), np.float32)
    p = np.arange(128)
    for t in range(3):
        r = 4 * t + p // 32
        consts[:, t] = -(T0 + HGRID * r) / HGRID           # 5.5 - r
        consts[:, 3 + t] = NR - r                           # 11 - r
    consts[:, 6] = (kan_bias.astype(np.float64)
                    + conv_bias.astype(np.float64)).astype(np.float32)
    return wt, consts


def kernel(x, spline_kernel, scale_factor, kan_bias, conv_bias):
    from concourse import bass_utils

    x = np.asarray(x, np.float32)
    spline_kernel = np.asarray(spline_kernel, np.float32)
    scale_factor = np.asarray(scale_factor, np.float32)
    kan_bias = np.asarray(kan_bias, np.float32)
    conv_bias = np.asarray(conv_bias, np.float32)

    key = f"nc_{MODE}"
    if key not in _cache:
        _cache[key] = _build_program(MODE)
    nc = _cache[key]

    wt, consts = _prep_static(MODE, spline_kernel, scale_factor,
                              kan_bias, conv_bias)

    in_maps = []
    for c in range(N_CORES):
        xc = x[c * BPC:(c + 1) * BPC]                      # (4,32,32,32)
        xtc = np.ascontiguousarray(
            xc.transpose(3, 0, 1, 2).reshape(C, BPC * PIX), np.float32
        )
        in_maps.append({"xt": xtc, "wt": wt, "consts": consts})

    res = bass_utils.run_bass_kernel_spmd(
        nc, in_maps, core_ids=list(range(N_CORES)),
        **_cache.get("run_kwargs", {})
    )
    _cache["last_result"] = res

    out = np.empty((B, HO, WO, F), np.float32)
    for c in range(N_CORES):
        yc = res.results[c]["y"]                           # (128, 3600)
        out[c * BPC:(c + 1) * BPC] = (
            yc.reshape(F, BPC, HO, WO).transpose(1, 2, 3, 0)
        )
    return out



# revision 4
# speedup vs baseline: 1.4409x; 1.4409x over previous
"""Conv2D-KAN Trainium2 kernel (8-core data-parallel SPMD), v2.

Formulation (per 3x3 patch, N = B*30*30 patches, in_size = 288):
    out[n,o] = sum_{i,k} sb[n,i,k] * (spline_kernel*scale)[i,k,o]
             + silu(xf) @ scale_factor + biases
with sb a cubic B-spline basis (8 funcs) over uniform knots
t_r = -2.2 + 0.4 r (r = 0..11, h = 0.4).

Device pipeline per image (4 images per core):
 1. x replicated to [128 = 4r x 32c, 1024 pix] (one DMA set).
 2. T_r = min(relu((x - t_r)/h), 11-r)^3 built in fp32
    (relu on ACT, clamp on DVE, square on ACT, cube-mul on DVE)
    as 3 tiles of [4r x 32c, 1024].
 3. The blend B_k = (1/6) sum_m cm_m T_{k+m} (cm = [1,-4,6,-4,1], /6
    folded into weights) needs fp32 T (large truncated cubes cancel),
    but fp32 matmuls run at 4 cyc/row. Instead T is split hi/lo:
    hi = bf16(T) (exact in PE bf16 mode), lo = T - hi (exact fp32
    Sterbenz difference, |lo| small so tf32 rounding is harmless).
    Combine = banded-matrix matmuls Ma/Mb against hi (bf16, 1 cyc/row)
    and lo (f32r, 1 cyc/row) accumulating in PSUM -> B tiles (f32r).
 4. silu: SL3 [96 = 3dj x 32c, 1024] holds silu(x) column-shifted by
    dj so the 9 silu conv offsets collapse into 3 matmuls of K=96.
 5. Main conv: per PSUM bank [128 filters, 450 patches]: 3 silu
    matmuls (K=96) + 18 basis matmuls (K=128, 9 offsets x 2 groups),
    all f32r at 1 cyc/row, then bias via ACT and DMA out.

Output [128, 3600] per core is transposed on host.
"""

import sys

sys.path.insert(0, "/opt/trn_rl_repo")

import numpy as np

N_CORES = 8
B, HH, WW, C = 32, 32, 32, 32
F = 128
KH = KW = 3
HO, WO = HH - KH + 1, WW - KW + 1          # 30, 30
BPC = B // N_CORES                          # images per core = 4
PIX = HH * WW                               # 1024 pixels per image
NPC = BPC * HO * WO                         # 3600 patches per core
BANKN = 450                                 # patches per psum bank
HGRID = 0.4
T0 = -2.2                                   # first knot
NR = 11

_cache = {}


def _build_program():
    import concourse.bacc as bacc
    import concourse.mybir as mybir
    import concourse.tile as tile

    f32 = mybir.dt.float32
    f32r = mybir.dt.float32r
    bf16 = mybir.dt.bfloat16
    AF = mybir.ActivationFunctionType
    ALU = mybir.AluOpType

    nc = bacc.Bacc("TRN2", target_bir_lowering=False, debug=False)
    xt = nc.dram_tensor("xt", [C, BPC * PIX], f32, kind="ExternalInput").ap()
    # basis chunk weights, chunk = g*9+off, rows k-major (4k x 32c)
    wb = nc.dram_tensor("wb", [128, 18 * F], f32r, kind="ExternalInput").ap()
    # silu chunk weights, chunk = di, rows 3dj x 32c
    ws = nc.dram_tensor("ws", [96, 3 * F], f32r, kind="ExternalInput").ap()
    # banded blend matrices Ma|Mb (cm values)
    wm = nc.dram_tensor("wm", [128, 2 * 128], f32, kind="ExternalInput").ap()
    consts = nc.dram_tensor("consts", [128, 8], f32, kind="ExternalInput").ap()
    y = nc.dram_tensor("y", [F, NPC], f32, kind="ExternalOutput").ap()

    with tile.TileContext(nc) as tc:
        with (
            tc.tile_pool(name="wp", bufs=1) as wp,
            tc.tile_pool(name="xp", bufs=2) as xp,
            tc.tile_pool(name="tp", bufs=2) as tp,
            tc.tile_pool(name="hp", bufs=2) as hp,
            tc.tile_pool(name="lp", bufs=2) as lp,
            tc.tile_pool(name="bp", bufs=2) as bp,
            tc.tile_pool(name="op", bufs=1) as op_,
            tc.tile_pool(name="pp", bufs=4, space="PSUM") as pp,
            tc.tile_pool(name="pb", bufs=3, space="PSUM") as pb,
        ):
            ct = wp.tile([128, 8], f32)
            nc.scalar.dma_start(ct[:], consts[:])

            # warm the ACT table set (silu's set carries relu / copy /
            # identity / square) before the first feature tile lands.
            warm = wp.tile([1, 1], f32, tag="warm")
            nc.scalar.activation(warm[:], ct[:1, :1], AF.Silu)

            # image 0's x-replica: four DMAs split across both HWDGE
            # queues ahead of all other traffic.
            xr0 = xp.tile([128, PIX], f32, tag="xr")
            eng = [nc.sync, nc.scalar, nc.sync, nc.scalar]
            for rep in range(4):
                eng[rep].dma_start(xr0[32 * rep:32 * rep + 32], xt[:, 0:PIX])

            # weights (gpsimd queue, overlaps image-0 feature build)
            wbt = wp.tile([128, 18 * F], f32r, tag="wb")
            nc.gpsimd.dma_start(wbt[:], wb[:])
            wst = wp.tile([96, 3 * F], f32r, tag="ws")
            nc.gpsimd.dma_start(wst[:], ws[:])
            wmt = wp.tile([128, 2 * 128], f32, tag="wm")
            nc.gpsimd.dma_start(wmt[:], wm[:])
            # blend matrices in both bf16 (for hi) and f32r (for lo)
            mB = wp.tile([128, 2 * 128], bf16, tag="mB")
            nc.vector.tensor_copy(mB[:], wmt[:])
            mR = wp.tile([128, 2 * 128], f32r, tag="mR")
            nc.vector.tensor_copy(mR[:], wmt[:])
            MaB, MbB = mB[:, 0:128], mB[:, 128:256]
            MaR, MbR = mR[:, 0:128], mR[:, 128:256]

            out_t = op_.tile([F, NPC], f32)

            for im in range(BPC):
                sl = slice(im * PIX, (im + 1) * PIX)
                if im == 0:
                    xr = xr0
                else:
                    xr = xp.tile([128, PIX], f32, tag="xr")
                    for rep in range(4):
                        eng[rep].dma_start(
                            xr[32 * rep:32 * rep + 32], xt[:, sl])

                # --- silu tile, dj-shifted into partition groups ---
                sl3 = xp.tile([96, PIX], f32r, tag="sl3")
                for g in range(3):
                    nc.scalar.activation(
                        sl3[32 * g:32 * g + 32, 0:PIX - g],
                        xr[0:32, g:PIX], AF.Silu)
                slv = sl3[:].rearrange("p (h w) -> p h w", w=WW)

                # --- T tiles + hi/lo split ---
                his, los = [], []
                for t in range(3):
                    T = tp.tile([128, PIX], f32, tag=f"T{t}")
                    nc.scalar.activation(
                        T[:], xr[:], AF.Relu,
                        bias=ct[:, t:t + 1], scale=1.0 / HGRID)
                    nc.vector.tensor_scalar_min(
                        T[:], T[:], ct[:, 3 + t:4 + t])
                    sq = tp.tile([128, PIX], f32, tag="sq")
                    nc.scalar.activation(sq[:], T[:], AF.Square)
                    nc.vector.tensor_mul(T[:], sq[:], T[:])
                    hi = hp.tile([128, PIX], bf16, tag=f"h{t}")
                    nc.gpsimd.tensor_copy(hi[:], T[:])
                    lo = lp.tile([128, PIX], f32r, tag=f"l{t}")
                    nc.gpsimd.tensor_sub(lo[:], T[:], hi[:])
                    his.append(hi)
                    los.append(lo)

                # --- blend B_k on PE (hi bf16 + lo f32r, 1 cyc/row) ---
                Bviews = []
                for g in range(2):
                    Bt = bp.tile([128, PIX], f32r, tag=f"B{g}")
                    for hf in range(2):
                        hs = slice(hf * 512, (hf + 1) * 512)
                        ps = pb.tile([128, 512], f32, tag="psB")
                        nc.tensor.matmul(ps[:], MaB, his[g][:, hs],
                                         start=True, stop=False)
                        nc.tensor.matmul(ps[:], MbB, his[g + 1][:, hs],
                                         start=False, stop=False)
                        nc.tensor.matmul(ps[:], MaR, los[g][:, hs],
                                         start=False, stop=False)
                        nc.tensor.matmul(ps[:], MbR, los[g + 1][:, hs],
                                         start=False, stop=True)
                        nc.vector.tensor_copy(Bt[:, hs], ps[:])
                    Bviews.append(
                        Bt[:].rearrange("p (h w) -> p h w", w=WW))

                # --- main conv: 2 banks x (3 silu + 18 basis) matmuls ---
                for half in range(2):
                    ps = pp.tile([F, BANKN], f32, tag="ps")
                    k = 0
                    for di in range(3):
                        h0 = half * 15 + di
                        nc.tensor.matmul(
                            ps[:], wst[:, di * F:(di + 1) * F],
                            slv[:, h0:h0 + 15, 0:WO],
                            start=(k == 0), stop=False)
                        k += 1
                    for g in range(2):
                        for off in range(9):
                            di, dj = divmod(off, KW)
                            h0 = half * 15 + di
                            nc.tensor.matmul(
                                ps[:],
                                wbt[:, (g * 9 + off) * F:(g * 9 + off + 1) * F],
                                Bviews[g][:, h0:h0 + 15, dj:dj + WO],
                                start=False, stop=(k == 20))
                            k += 1
                    s = (im * 2 + half) * BANKN
                    nc.scalar.activation(
                        out_t[:, s:s + BANKN], ps[:], AF.Identity,
                        bias=ct[:, 6:7], scale=1.0)
                    nc.sync.dma_start(y[:, s:s + BANKN], out_t[:, s:s + BANKN])

    nc.compile()
    return nc


def _prep_static(spline_kernel, scale_factor, kan_bias, conv_bias):
    w6 = (spline_kernel.astype(np.float64)
          * scale_factor.astype(np.float64)[:, None, :]) / 6.0
    w6 = w6.reshape(KH * KW, C, 8, F)
    Wb = np.zeros((18, 128, F), np.float64)
    for off in range(9):
        for g in range(2):
            blk = w6[off][:, 4 * g:4 * g + 4]            # (32c, 4k, F)
            Wb[g * 9 + off] = blk.transpose(1, 0, 2).reshape(128, F)
    wb = np.ascontiguousarray(
        Wb.transpose(1, 0, 2).reshape(128, 18 * F), np.float32)

    sf9 = scale_factor.astype(np.float64).reshape(9, C, F)
    Ws = np.zeros((3, 96, F), np.float64)
    for di in range(3):
        Ws[di] = sf9[3 * di:3 * di + 3].reshape(96, F)
    ws = np.ascontiguousarray(
        Ws.transpose(1, 0, 2).reshape(96, 3 * F), np.float32)

    cm = np.array([1.0, -4.0, 6.0, -4.0, 1.0])
    pin = np.arange(128)[:, None]
    pout = np.arange(128)[None, :]
    same_c = (pin % 32) == (pout % 32)
    Ms = []
    for base in (0, 4):
        m = base + pin // 32 - pout // 32
        Ms.append(np.where((m >= 0) & (m <= 4) & same_c,
                           cm[np.clip(m, 0, 4)], 0.0))
    wm = np.ascontiguousarray(
        np.concatenate(Ms, axis=1), np.float32)          # [128, 256]

    consts = np.zeros((128, 8), np.float32)
    p = np.arange(128)
    for t in range(3):
        r = 4 * t + p // 32
        consts[:, t] = -(T0 + HGRID * r) / HGRID          # 5.5 - r
        consts[:, 3 + t] = NR - r                          # 11 - r
    consts[:, 6] = (kan_bias.astype(np.float64)
                    + conv_bias.astype(np.float64)).astype(np.float32)
    return wb, ws, wm, consts


def kernel(x, spline_kernel, scale_factor, kan_bias, conv_bias):
    from concourse import bass_utils

    x = np.asarray(x, np.float32)
    spline_kernel = np.asarray(spline_kernel, np.float32)
    scale_factor = np.asarray(scale_factor, np.float32)
    kan_bias = np.asarray(kan_bias, np.float32)
    conv_bias = np.asarray(conv_bias, np.float32)

    if "nc" not in _cache:
        _cache["nc"] = _build_program()
    nc = _cache["nc"]

    wb, ws, wm, consts = _prep_static(
        spline_kernel, scale_factor, kan_bias, conv_bias)

    in_maps = []
    for c in range(N_CORES):
        xc = x[c * BPC:(c + 1) * BPC]                     # (4,32,32,32)
        xtc = np.ascontiguousarray(
            xc.transpose(3, 0, 1, 2).reshape(C, BPC * PIX), np.float32
        )
        in_maps.append(
            {"xt": xtc, "wb": wb, "ws": ws, "wm": wm, "consts": consts})

    res = bass_utils.run_bass_kernel_spmd(
        nc, in_maps, core_ids=list(range(N_CORES)),
        **_cache.get("run_kwargs", {})
    )
    _cache["last_result"] = res

    out = np.empty((B, HO, WO, F), np.float32)
    for c in range(N_CORES):
        yc = res.results[c]["y"]                          # (128, 3600)
        out[c * BPC:(c + 1) * BPC] = (
            yc.reshape(F, BPC, HO, WO).transpose(1, 2, 3, 0)
        )
    return out


# revision 8
# speedup vs baseline: 2.1831x; 1.5151x over previous
"""Conv2D-KAN Trainium2 kernel (8-core data-parallel SPMD), v3.

Formulation (per 3x3 patch, N = B*30*30 patches, in_size = 288):
    out[n,o] = sum_{i,k} sb[n,i,k] * (spline_kernel*scale)[i,k,o]
             + silu(xf) @ scale_factor + biases
with sb a cubic B-spline basis (8 funcs) over uniform knots
t_r = -2.2 + 0.4 r (r = 0..11, h = 0.4).

Device pipeline per image (4 images per core):
 1. x replicated to [128 = 4r x 32c, 1024 pix] (one DMA set).
 2. T_r = relu((x - t_r)/h)^3 in fp32 (relu+square on ACT, cube-mul
    on DVE), 3 tiles of [4r x 32c, 1024]. No clamp: the 4th-difference
    identity sum_m cm_m T_{k+m} = 6 B_k holds unclamped everywhere
    (it vanishes identically right of each basis support).
 3. The blend needs fp32 T (large cubes cancel), but fp32 matmuls run
    at 4 cyc/row. T is split hi/lo: hi = bf16(T) (exact in PE bf16
    mode), lo = T - hi (exact Sterbenz difference; |lo| <= ulp so
    tf32 rounding of it is harmless). Blend = banded matrices Ma/Mb
    (cm = [1,-4,6,-4,1], /6 folded into weights) against hi (bf16)
    and lo (f32r), 1 cyc/row, stationary-major over 4 PSUM banks so
    each matrix loads into the PE once per image -> B tiles (bf16).
 4. silu: SL3 [96 = 3dj x 32c, 1024] holds silu(x) column-shifted by
    dj so the 9 silu conv offsets collapse into 3 matmuls of K=96.
 5. Main conv, weight-stationary: 21 chunks (3 silu K=96 + 18 basis
    K=128), each loaded once and streamed into both PSUM banks
    [128 filters, 450 patches] of the image, all bf16 at 1 cyc/row,
    then bias via ACT and DMA out.

Output [128, 3600] per core is transposed on host.
"""

import sys

sys.path.insert(0, "/opt/trn_rl_repo")

import numpy as np

N_CORES = 8
B, HH, WW, C = 32, 32, 32, 32
F = 128
KH = KW = 3
HO, WO = HH - KH + 1, WW - KW + 1          # 30, 30
BPC = B // N_CORES                          # images per core = 4
PIX = HH * WW                               # 1024 pixels per image
NPC = BPC * HO * WO                         # 3600 patches per core
BANKN = 450                                 # patches per psum bank
HGRID = 0.4
T0 = -2.2                                   # first knot
NR = 11

_cache = {}


def _build_program():
    import concourse.bacc as bacc
    import concourse.mybir as mybir
    import concourse.tile as tile

    f32 = mybir.dt.float32
    f32r = mybir.dt.float32r
    bf16 = mybir.dt.bfloat16
    AF = mybir.ActivationFunctionType

    nc = bacc.Bacc("TRN2", target_bir_lowering=False, debug=False)
    xt = nc.dram_tensor("xt", [C, BPC * PIX], f32, kind="ExternalInput").ap()
    # basis chunk weights, chunk = g*9+off, rows k-major (4k x 32c)
    wb = nc.dram_tensor("wb", [128, 18 * F], bf16, kind="ExternalInput").ap()
    # silu chunk weights, chunk = di, rows 3dj x 32c
    ws = nc.dram_tensor("ws", [96, 3 * F], bf16, kind="ExternalInput").ap()
    # banded blend matrices Ma|Mb (cm values)
    wm = nc.dram_tensor("wm", [128, 2 * 128], f32, kind="ExternalInput").ap()
    consts = nc.dram_tensor("consts", [128, 8], f32, kind="ExternalInput").ap()
    y = nc.dram_tensor("y", [F, NPC], f32, kind="ExternalOutput").ap()

    with tile.TileContext(nc) as tc:
        with (
            tc.tile_pool(name="wp", bufs=1) as wp,
            tc.tile_pool(name="xp", bufs=2) as xp,
            tc.tile_pool(name="tp", bufs=2) as tp,
            tc.tile_pool(name="hp", bufs=2) as hp,
            tc.tile_pool(name="lp", bufs=2) as lp,
            tc.tile_pool(name="bp", bufs=2) as bp,
            tc.tile_pool(name="op", bufs=1) as op_,
            tc.tile_pool(name="pp", bufs=2, space="PSUM") as pp,
            tc.tile_pool(name="pb", bufs=1, space="PSUM") as pb,
        ):
            ct = wp.tile([128, 8], f32)
            nc.scalar.dma_start(ct[:], consts[:])

            # warm the ACT table set (silu's set carries relu / copy /
            # identity / square) before the first feature tile lands.
            warm = wp.tile([1, 1], f32, tag="warm")
            nc.scalar.activation(warm[:], ct[:1, :1], AF.Silu)

            # image 0's x-replica: four DMAs split across two queues
            # ahead of all other traffic.
            xr0 = xp.tile([128, PIX], f32, tag="xr")
            eng = [nc.sync, nc.gpsimd, nc.sync, nc.gpsimd]
            for rep in range(4):
                eng[rep].dma_start(xr0[32 * rep:32 * rep + 32], xt[:, 0:PIX])

            # weights (scalar queue: idle until image 0 features exist)
            wbt = wp.tile([128, 18 * F], bf16, tag="wb")
            nc.scalar.dma_start(wbt[:], wb[:])
            wst = wp.tile([96, 3 * F], bf16, tag="ws")
            nc.scalar.dma_start(wst[:], ws[:])
            wmt = wp.tile([128, 2 * 128], f32, tag="wm")
            nc.scalar.dma_start(wmt[:], wm[:])
            # blend matrices in bf16 (for hi) and f32r (for lo)
            mB = wp.tile([128, 2 * 128], bf16, tag="mB")
            nc.vector.tensor_copy(mB[:], wmt[:])
            mR = wp.tile([128, 2 * 128], f32r, tag="mR")
            nc.vector.tensor_copy(mR[:], wmt[:])
            stat_hi = [mB[:, 0:128], mB[:, 128:256]]       # MaB, MbB
            stat_lo = [mR[:, 0:128], mR[:, 128:256]]       # MaR, MbR

            out_t = op_.tile([F, NPC], f32)

            for im in range(BPC):
                sl = slice(im * PIX, (im + 1) * PIX)
                if im == 0:
                    xr = xr0
                else:
                    xr = xp.tile([128, PIX], f32, tag="xr")
                    for rep in range(4):
                        eng[rep].dma_start(
                            xr[32 * rep:32 * rep + 32], xt[:, sl])

                # --- silu tile, dj-shifted into partition groups ---
                sl3 = xp.tile([96, PIX], bf16, tag="sl3")
                for g in range(3):
                    nc.scalar.activation(
                        sl3[32 * g:32 * g + 32, 0:PIX - g],
                        xr[0:32, g:PIX], AF.Silu)
                slv = sl3[:].rearrange("p (h w) -> p h w", w=WW)

                # --- T tiles + hi/lo split ---
                his, los = [], []
                for t in range(3):
                    T = tp.tile([128, PIX], f32, tag=f"T{t}")
                    nc.scalar.activation(
                        T[:], xr[:], AF.Relu,
                        bias=ct[:, t:t + 1], scale=1.0 / HGRID)
                    sq = tp.tile([128, PIX], f32, tag="sq")
                    nc.scalar.activation(sq[:], T[:], AF.Square)
                    nc.vector.tensor_mul(T[:], sq[:], T[:])
                    hi = hp.tile([128, PIX], bf16, tag=f"h{t}")
                    nc.vector.tensor_copy(hi[:], T[:])
                    lo = lp.tile([128, PIX], f32r, tag=f"l{t}")
                    nc.gpsimd.tensor_sub(lo[:], T[:], hi[:])
                    his.append(hi)
                    los.append(lo)

                # --- blend B_k on PE, stationary-major (4 LDW/image) ---
                pbs = [[pb.tile([128, 512], f32, tag=f"pb{g}{hf}",
                                name=f"pb{g}{hf}")
                        for hf in range(2)] for g in range(2)]
                for si, (stat, srcs) in enumerate((
                        (stat_hi[0], his), (stat_hi[1], his),
                        (stat_lo[0], los), (stat_lo[1], los))):
                    d = si % 2                             # Ma: g, Mb: g+1
                    for g in range(2):
                        for hf in range(2):
                            hs = slice(hf * 512, (hf + 1) * 512)
                            nc.tensor.matmul(
                                pbs[g][hf][:], stat, srcs[g + d][:, hs],
                                start=(si == 0), stop=(si == 3))
                Bviews = []
                for g in range(2):
                    Bt = bp.tile([128, PIX], bf16, tag=f"B{g}")
                    for hf in range(2):
                        hs = slice(hf * 512, (hf + 1) * 512)
                        nc.vector.tensor_copy(Bt[:, hs], pbs[g][hf][:])
                    Bviews.append(
                        Bt[:].rearrange("p (h w) -> p h w", w=WW))

                # --- main conv, weight-stationary over both banks ---
                pss = [pp.tile([F, BANKN], f32, tag=f"ps{hf}",
                               name=f"ps{hf}")
                       for hf in range(2)]
                for ci in range(21):
                    if ci < 3:
                        di = ci
                        lhsT = wst[:, di * F:(di + 1) * F]
                        rhs = [slv[:, half * 15 + di:half * 15 + di + 15, 0:WO]
                               for half in range(2)]
                    else:
                        g, off = divmod(ci - 3, 9)
                        di, dj = divmod(off, KW)
                        lhsT = wbt[:, (g * 9 + off) * F:(g * 9 + off + 1) * F]
                        rhs = [Bviews[g][:, half * 15 + di:half * 15 + di + 15,
                                         dj:dj + WO]
                               for half in range(2)]
                    for half in range(2):
                        nc.tensor.matmul(
                            pss[half][:], lhsT, rhs[half],
                            start=(ci == 0), stop=(ci == 20))
                for half in range(2):
                    s = (im * 2 + half) * BANKN
                    nc.scalar.activation(
                        out_t[:, s:s + BANKN], pss[half][:], AF.Identity,
                        bias=ct[:, 6:7], scale=1.0)
                    nc.sync.dma_start(y[:, s:s + BANKN], out_t[:, s:s + BANKN])

    nc.compile()
    return nc


def _prep_static(spline_kernel, scale_factor, kan_bias, conv_bias):
    import ml_dtypes

    w6 = (spline_kernel.astype(np.float64)
          * scale_factor.astype(np.float64)[:, None, :]) / 6.0
    w6 = w6.reshape(KH * KW, C, 8, F)
    Wb = np.zeros((18, 128, F), np.float64)
    for off in range(9):
        for g in range(2):
            blk = w6[off][:, 4 * g:4 * g + 4]            # (32c, 4k, F)
            Wb[g * 9 + off] = blk.transpose(1, 0, 2).reshape(128, F)
    wb = np.ascontiguousarray(
        Wb.transpose(1, 0, 2).reshape(128, 18 * F)).astype(ml_dtypes.bfloat16)

    sf9 = scale_factor.astype(np.float64).reshape(9, C, F)
    Ws = np.zeros((3, 96, F), np.float64)
    for di in range(3):
        Ws[di] = sf9[3 * di:3 * di + 3].reshape(96, F)
    ws = np.ascontiguousarray(
        Ws.transpose(1, 0, 2).reshape(96, 3 * F)).astype(ml_dtypes.bfloat16)

    cm = np.array([1.0, -4.0, 6.0, -4.0, 1.0])
    pin = np.arange(128)[:, None]
    pout = np.arange(128)[None, :]
    same_c = (pin % 32) == (pout % 32)
    Ms = []
    for base in (0, 4):
        m = base + pin // 32 - pout // 32
        Ms.append(np.where((m >= 0) & (m <= 4) & same_c,
                           cm[np.clip(m, 0, 4)], 0.0))
    wm = np.ascontiguousarray(
        np.concatenate(Ms, axis=1), np.float32)          # [128, 256]

    consts = np.zeros((128, 8), np.float32)
    p = np.arange(128)
    for t in range(3):
        r = 4 * t + p // 32
        consts[:, t] = -(T0 + HGRID * r) / HGRID          # 5.5 - r
    consts[:, 6] = (kan_bias.astype(np.float64)
                    + conv_bias.astype(np.float64)).astype(np.float32)
    return wb, ws, wm, consts


def kernel(x, spline_kernel, scale_factor, kan_bias, conv_bias):
    from concourse import bass_utils

    x = np.asarray(x, np.float32)
    spline_kernel = np.asarray(spline_kernel, np.float32)
    scale_factor = np.asarray(scale_factor, np.float32)
    kan_bias = np.asarray(kan_bias, np.float32)
    conv_bias = np.asarray(conv_bias, np.float32)

    if "nc" not in _cache:
        _cache["nc"] = _build_program()
    nc = _cache["nc"]

    wb, ws, wm, consts = _prep_static(
        spline_kernel, scale_factor, kan_bias, conv_bias)

    in_maps = []
    for c in range(N_CORES):
        xc = x[c * BPC:(c + 1) * BPC]                     # (4,32,32,32)
        xtc = np.ascontiguousarray(
            xc.transpose(3, 0, 1, 2).reshape(C, BPC * PIX), np.float32
        )
        in_maps.append(
            {"xt": xtc, "wb": wb, "ws": ws, "wm": wm, "consts": consts})

    res = bass_utils.run_bass_kernel_spmd(
        nc, in_maps, core_ids=list(range(N_CORES)),
        **_cache.get("run_kwargs", {})
    )
    _cache["last_result"] = res

    out = np.empty((B, HO, WO, F), np.float32)
    for c in range(N_CORES):
        yc = res.results[c]["y"]                          # (128, 3600)
        out[c * BPC:(c + 1) * BPC] = (
            yc.reshape(F, BPC, HO, WO).transpose(1, 2, 3, 0)
        )
    return out


# revision 9
# speedup vs baseline: 2.2042x; 1.0096x over previous
"""Conv2D-KAN Trainium2 kernel (8-core data-parallel SPMD), v3.

Formulation (per 3x3 patch, N = B*30*30 patches, in_size = 288):
    out[n,o] = sum_{i,k} sb[n,i,k] * (spline_kernel*scale)[i,k,o]
             + silu(xf) @ scale_factor + biases
with sb a cubic B-spline basis (8 funcs) over uniform knots
t_r = -2.2 + 0.4 r (r = 0..11, h = 0.4).

Device pipeline per image (4 images per core):
 1. x replicated to [128 = 4r x 32c, 1024 pix] (one DMA set).
 2. T_r = relu((x - t_r)/h)^3 in fp32 (relu+square on ACT, cube-mul
    on DVE), 3 tiles of [4r x 32c, 1024]. No clamp: the 4th-difference
    identity sum_m cm_m T_{k+m} = 6 B_k holds unclamped everywhere
    (it vanishes identically right of each basis support).
 3. The blend needs fp32 T (large cubes cancel), but fp32 matmuls run
    at 4 cyc/row. T is split hi/lo: hi = bf16(T) (exact in PE bf16
    mode), lo = T - hi (exact Sterbenz difference; |lo| <= ulp so
    tf32 rounding of it is harmless). Blend = banded matrices Ma/Mb
    (cm = [1,-4,6,-4,1], /6 folded into weights) against hi (bf16)
    and lo (f32r), 1 cyc/row, stationary-major over 4 PSUM banks so
    each matrix loads into the PE once per image -> B tiles (bf16).
 4. silu: SL3 [96 = 3dj x 32c, 1024] holds silu(x) column-shifted by
    dj so the 9 silu conv offsets collapse into 3 matmuls of K=96.
 5. Main conv, weight-stationary: 21 chunks (3 silu K=96 + 18 basis
    K=128), each loaded once and streamed into both PSUM banks
    [128 filters, 450 patches] of the image, all bf16 at 1 cyc/row,
    then bias via ACT and DMA out.

Output [128, 3600] per core is transposed on host.
"""

import sys

sys.path.insert(0, "/opt/trn_rl_repo")

import numpy as np

N_CORES = 8
B, HH, WW, C = 32, 32, 32, 32
F = 128
KH = KW = 3
HO, WO = HH - KH + 1, WW - KW + 1          # 30, 30
BPC = B // N_CORES                          # images per core = 4
PIX = HH * WW                               # 1024 pixels per image
NPC = BPC * HO * WO                         # 3600 patches per core
BANKN = 450                                 # patches per psum bank
HGRID = 0.4
T0 = -2.2                                   # first knot
NR = 11

_cache = {}


def _build_program():
    import concourse.bacc as bacc
    import concourse.mybir as mybir
    import concourse.tile as tile

    f32 = mybir.dt.float32
    f32r = mybir.dt.float32r
    bf16 = mybir.dt.bfloat16
    f16 = mybir.dt.float16
    AF = mybir.ActivationFunctionType

    nc = bacc.Bacc("TRN2", target_bir_lowering=False, debug=False)
    xt = nc.dram_tensor("xt", [C, BPC * PIX], f32, kind="ExternalInput").ap()
    # basis chunk weights, chunk = g*9+off, rows k-major (4k x 32c)
    wb = nc.dram_tensor("wb", [128, 18 * F], bf16, kind="ExternalInput").ap()
    # silu chunk weights, chunk = di, rows 3dj x 32c
    ws = nc.dram_tensor("ws", [96, 3 * F], bf16, kind="ExternalInput").ap()
    # banded blend matrices Ma|Mb (cm values)
    wm = nc.dram_tensor("wm", [128, 2 * 128], f32, kind="ExternalInput").ap()
    consts = nc.dram_tensor("consts", [128, 8], f32, kind="ExternalInput").ap()
    y = nc.dram_tensor("y", [F, NPC], f32, kind="ExternalOutput").ap()

    with tile.TileContext(nc) as tc:
        with (
            tc.tile_pool(name="wp", bufs=1) as wp,
            tc.tile_pool(name="xp", bufs=2) as xp,
            tc.tile_pool(name="tp", bufs=2) as tp,
            tc.tile_pool(name="hp", bufs=2) as hp,
            tc.tile_pool(name="lp", bufs=2) as lp,
            tc.tile_pool(name="bp", bufs=2) as bp,
            tc.tile_pool(name="op", bufs=1) as op_,
            tc.tile_pool(name="pp", bufs=2, space="PSUM") as pp,
            tc.tile_pool(name="pb", bufs=1, space="PSUM") as pb,
        ):
            ct = wp.tile([128, 8], f32)
            nc.scalar.dma_start(ct[:], consts[:])

            # warm the ACT table set (silu's set carries relu / copy /
            # identity / square) before the first feature tile lands.
            warm = wp.tile([1, 1], f32, tag="warm")
            nc.scalar.activation(warm[:], ct[:1, :1], AF.Silu)

            # image 0's x-replica: four DMAs split across two queues
            # ahead of all other traffic.
            xr0 = xp.tile([128, PIX], f32, tag="xr")
            eng = [nc.sync, nc.gpsimd, nc.sync, nc.gpsimd]
            for rep in range(4):
                eng[rep].dma_start(xr0[32 * rep:32 * rep + 32], xt[:, 0:PIX])

            # weights (scalar queue: idle until image 0 features exist)
            wbt = wp.tile([128, 18 * F], bf16, tag="wb")
            nc.scalar.dma_start(wbt[:], wb[:])
            wst = wp.tile([96, 3 * F], bf16, tag="ws")
            nc.scalar.dma_start(wst[:], ws[:])
            wmt = wp.tile([128, 2 * 128], f32, tag="wm")
            nc.scalar.dma_start(wmt[:], wm[:])
            # blend matrices in fp16 (hi and lo both fp16, 1 cyc/row)
            mF = wp.tile([128, 2 * 128], f16, tag="mF")
            nc.vector.tensor_copy(mF[:], wmt[:])
            stats = [mF[:, 0:128], mF[:, 128:256]]         # Ma, Mb

            out_t = op_.tile([F, NPC], f32)

            for im in range(BPC):
                sl = slice(im * PIX, (im + 1) * PIX)
                if im == 0:
                    xr = xr0
                else:
                    xr = xp.tile([128, PIX], f32, tag="xr")
                    for rep in range(4):
                        eng[rep].dma_start(
                            xr[32 * rep:32 * rep + 32], xt[:, sl])

                # --- silu tile, dj-shifted into partition groups ---
                sl3 = xp.tile([96, PIX], bf16, tag="sl3")
                for g in range(3):
                    nc.scalar.activation(
                        sl3[32 * g:32 * g + 32, 0:PIX - g],
                        xr[0:32, g:PIX], AF.Silu)
                slv = sl3[:].rearrange("p (h w) -> p h w", w=WW)

                # --- T tiles + hi/lo split ---
                his, los = [], []
                for t in range(3):
                    T = tp.tile([128, PIX], f32, tag=f"T{t}")
                    nc.scalar.activation(
                        T[:], xr[:], AF.Relu,
                        bias=ct[:, t:t + 1], scale=1.0 / HGRID)
                    sq = tp.tile([128, PIX], f32, tag="sq")
                    nc.scalar.activation(sq[:], T[:], AF.Square)
                    nc.vector.tensor_mul(T[:], sq[:], T[:])
                    hi = hp.tile([128, PIX], f16, tag=f"h{t}")
                    nc.vector.tensor_copy(hi[:], T[:])
                    lo = lp.tile([128, PIX], f16, tag=f"l{t}")
                    nc.gpsimd.tensor_sub(lo[:], T[:], hi[:])
                    his.append(hi)
                    los.append(lo)

                # --- blend B_k on PE, stationary-major (4 LDW/image) ---
                pbs = [[pb.tile([128, 512], f32, tag=f"pb{g}{hf}",
                                name=f"pb{g}{hf}")
                        for hf in range(2)] for g in range(2)]
                for si, (stat, srcs, d) in enumerate((
                        (stats[0], his, 0), (stats[0], los, 0),
                        (stats[1], his, 1), (stats[1], los, 1))):
                    for g in range(2):
                        for hf in range(2):
                            hs = slice(hf * 512, (hf + 1) * 512)
                            nc.tensor.matmul(
                                pbs[g][hf][:], stat, srcs[g + d][:, hs],
                                start=(si == 0), stop=(si == 3))
                Bviews = []
                for g in range(2):
                    Bt = bp.tile([128, PIX], bf16, tag=f"B{g}")
                    for hf in range(2):
                        hs = slice(hf * 512, (hf + 1) * 512)
                        nc.vector.tensor_copy(Bt[:, hs], pbs[g][hf][:])
                    Bviews.append(
                        Bt[:].rearrange("p (h w) -> p h w", w=WW))

                # --- main conv, weight-stationary over both banks ---
                pss = [pp.tile([F, BANKN], f32, tag=f"ps{hf}",
                               name=f"ps{hf}")
                       for hf in range(2)]
                for ci in range(21):
                    if ci < 3:
                        di = ci
                        lhsT = wst[:, di * F:(di + 1) * F]
                        rhs = [slv[:, half * 15 + di:half * 15 + di + 15, 0:WO]
                               for half in range(2)]
                    else:
                        g, off = divmod(ci - 3, 9)
                        di, dj = divmod(off, KW)
                        lhsT = wbt[:, (g * 9 + off) * F:(g * 9 + off + 1) * F]
                        rhs = [Bviews[g][:, half * 15 + di:half * 15 + di + 15,
                                         dj:dj + WO]
                               for half in range(2)]
                    for half in range(2):
                        nc.tensor.matmul(
                            pss[half][:], lhsT, rhs[half],
                            start=(ci == 0), stop=(ci == 20))
                for half in range(2):
                    s = (im * 2 + half) * BANKN
                    nc.scalar.activation(
                        out_t[:, s:s + BANKN], pss[half][:], AF.Identity,
                        bias=ct[:, 6:7], scale=1.0)
                    nc.sync.dma_start(y[:, s:s + BANKN], out_t[:, s:s + BANKN])

    nc.compile()
    return nc


def _prep_static(spline_kernel, scale_factor, kan_bias, conv_bias):
    import ml_dtypes

    w6 = (spline_kernel.astype(np.float64)
          * scale_factor.astype(np.float64)[:, None, :]) / 6.0
    w6 = w6.reshape(KH * KW, C, 8, F)
    Wb = np.zeros((18, 128, F), np.float64)
    for off in range(9):
        for g in range(2):
            blk = w6[off][:, 4 * g:4 * g + 4]            # (32c, 4k, F)
            Wb[g * 9 + off] = blk.transpose(1, 0, 2).reshape(128, F)
    wb = np.ascontiguousarray(
        Wb.transpose(1, 0, 2).reshape(128, 18 * F)).astype(ml_dtypes.bfloat16)

    sf9 = scale_factor.astype(np.float64).reshape(9, C, F)
    Ws = np.zeros((3, 96, F), np.float64)
    for di in range(3):
        Ws[di] = sf9[3 * di:3 * di + 3].reshape(96, F)
    ws = np.ascontiguousarray(
        Ws.transpose(1, 0, 2).reshape(96, 3 * F)).astype(ml_dtypes.bfloat16)

    cm = np.array([1.0, -4.0, 6.0, -4.0, 1.0])
    pin = np.arange(128)[:, None]
    pout = np.arange(128)[None, :]
    same_c = (pin % 32) == (pout % 32)
    Ms = []
    for base in (0, 4):
        m = base + pin // 32 - pout // 32
        Ms.append(np.where((m >= 0) & (m <= 4) & same_c,
                           cm[np.clip(m, 0, 4)], 0.0))
    wm = np.ascontiguousarray(
        np.concatenate(Ms, axis=1), np.float32)          # [128, 256]

    consts = np.zeros((128, 8), np.float32)
    p = np.arange(128)
    for t in range(3):
        r = 4 * t + p // 32
        consts[:, t] = -(T0 + HGRID * r) / HGRID          # 5.5 - r
    consts[:, 6] = (kan_bias.astype(np.float64)
                    + conv_bias.astype(np.float64)).astype(np.float32)
    return wb, ws, wm, consts


def kernel(x, spline_kernel, scale_factor, kan_bias, conv_bias):
    from concourse import bass_utils

    x = np.asarray(x, np.float32)
    spline_kernel = np.asarray(spline_kernel, np.float32)
    scale_factor = np.asarray(scale_factor, np.float32)
    kan_bias = np.asarray(kan_bias, np.float32)
    conv_bias = np.asarray(conv_bias, np.float32)

    if "nc" not in _cache:
        _cache["nc"] = _build_program()
    nc = _cache["nc"]

    wb, ws, wm, consts = _prep_static(
        spline_kernel, scale_factor, kan_bias, conv_bias)

    in_maps = []
    for c in range(N_CORES):
        xc = x[c * BPC:(c + 1) * BPC]                     # (4,32,32,32)
        xtc = np.ascontiguousarray(
            xc.transpose(3, 0, 1, 2).reshape(C, BPC * PIX), np.float32
        )
        in_maps.append(
            {"xt": xtc, "wb": wb, "ws": ws, "wm": wm, "consts": consts})

    res = bass_utils.run_bass_kernel_spmd(
        nc, in_maps, core_ids=list(range(N_CORES)),
        **_cache.get("run_kwargs", {})
    )
    _cache["last_result"] = res

    out = np.empty((B, HO, WO, F), np.float32)
    for c in range(N_CORES):
        yc = res.results[c]["y"]                          # (128, 3600)
        out[c * BPC:(c + 1) * BPC] = (
            yc.reshape(F, BPC, HO, WO).transpose(1, 2, 3, 0)
        )
    return out
